# revision 33
# baseline (speedup 1.0000x reference)
"""GNN edge-MLP classifier kernel for 8 Trainium2 NeuronCores.

Reference computation (per edge e):
    x = [x_student[src[e]], edge_feat[e], x_item[dst[e]]]   # [320]
    h = elu(x @ W1 + b1)                                    # [256]
    out[e] = h @ W2 + b2 + offset[dst[e]]

Sharding: edges split 8-way (data parallel); node tables + weights
replicated per core. No collectives needed (forward only).

Device strategy per core (v2):
  - Host sorts its edge shard into 16 classes by (src//32768, dst//32768)
    so node-table gathers can use int16 indices against a per-class base
    offset (dma_gather transpose-mode custom instruction).
  - Both node-table gathers move the minimum 256B/edge (elem=128 bf16);
    offset[dst] no longer rides in the x_item gather — the per-edge
    offset values are pre-gathered on host (like the edge sort itself)
    and stream in sequentially as an fp32 side channel.
  - b1 is folded into the W1 matmul via an all-ones row appended to the
    streamed edge features (efT row 64) matched with a b1 row in W1.
  - ELU is restructured to skip the explicit elu tile:
        elu(h) @ W2 = relu(h) @ W2 + min(exp(h),1) @ W2 - sum(W2)
    (exact: for h>0 the min()==1 term cancels with -sum(W2)).  The
    -sum(W2)+b2 constant is folded into the host offset stream, and the
    final add runs as one DVE tensor_tensor that also evacuates PSUM.
  - ~16 instructions per 512-edge window: 2 gathers, 10 matmuls,
    2 activations (exp), 3-4 vector ops.
  - Gathers run free (no serializing dep chain) on 2 SWDGE queues with
    DMA transfers overlapping desc-gen; see the comment in _build for the
    concurrency/aliasing safety argument.
"""
import sys
sys.path.insert(0, "/opt/trn_rl_repo")
from contextlib import ExitStack

import numpy as np
import ml_dtypes

import concourse.bass as bass
from concourse import bacc
import concourse.mybir as mybir
import concourse.tile as tile
from concourse.bass_utils import run_bass_kernel_spmd

N_NODES = 100000
N_EDGES = 1000000
IN_CH = 128
EDGE_DIM = 64
DEC_CH = 256
N_CORES = 8
E_PER = N_EDGES // N_CORES
BUCKET = 32768
N_BKT = (N_NODES + BUCKET - 1) // BUCKET  # 4
WIN = 512           # psum window (edges per matmul group)
BLK = 4096          # max edges per gather instruction

BF16 = ml_dtypes.bfloat16


# ---------------------------------------------------------------- host prep

def _class_ids(src, dst):
    return (src // BUCKET) * N_BKT + (dst // BUCKET)


def _prep_cores(src_all, dst_all, ef_all, off_all):
    """Sort each core's edges by (src,dst) bucket class; pad classes to a
    uniform per-class capacity so one SPMD program fits all cores."""
    shards = []
    counts = np.zeros((N_CORES, N_BKT * N_BKT), np.int64)
    for c in range(N_CORES):
        s = slice(c * E_PER, (c + 1) * E_PER)
        src, dst = src_all[s], dst_all[s]
        cls = _class_ids(src, dst)
        order = np.argsort(cls, kind="stable")
        shards.append((src, dst, ef_all[s], cls, order))
        counts[c] = np.bincount(cls, minlength=N_BKT * N_BKT)

    caps = counts.max(axis=0)
    caps = ((caps + WIN - 1) // WIN) * WIN  # pad each class to 512-mult
    e_tot = int(caps.sum())

    blocks = []  # (offset, n, bs, bd)
    a = 0
    for k in range(N_BKT * N_BKT):
        cap = int(caps[k])
        while cap > 0:
            n = min(BLK, cap)
            blocks.append((a, n, k // N_BKT, k % N_BKT))
            a += n
            cap -= n

    per_core = []
    for c in range(N_CORES):
        src, dst, ef, cls, order = shards[c]
        idx_fs = np.zeros(e_tot, np.int16)
        idx_fi = np.zeros(e_tot, np.int16)
        efp = np.zeros((e_tot, EDGE_DIM), np.float32)
        offe = np.zeros(e_tot, np.float32)
        pos = np.full(e_tot, -1, np.int64)
        a = 0
        cls_sorted = cls[order]
        for k in range(N_BKT * N_BKT):
            sel = order[np.searchsorted(cls_sorted, k):
                        np.searchsorted(cls_sorted, k + 1)]
            nk = len(sel)
            idx_fs[a:a + nk] = (src[sel] - (k // N_BKT) * BUCKET).astype(np.int16)
            idx_fi[a:a + nk] = (dst[sel] - (k % N_BKT) * BUCKET).astype(np.int16)
            efp[a:a + nk] = ef[sel]
            offe[a:a + nk] = off_all[dst[sel]]
            pos[a:a + nk] = sel
            a += int(caps[k])

        def wrap(ii):
            w = ii.reshape(-1, 16).T.copy()          # [16, e_tot/16]
            return np.tile(w, (8, 1))                 # [128, e_tot/16]

        eftr = np.ones((EDGE_DIM + 1, e_tot), np.float32)
        eftr[:EDGE_DIM] = efp.T
        per_core.append({
            "idx_fs": wrap(idx_fs),
            "idx_fi": wrap(idx_fi),
            "efT": np.ascontiguousarray(eftr).astype(BF16),  # [65, e_tot]
            "offE": offe.reshape(1, -1),
            "pos": pos,
        })
    return blocks, e_tot, per_core


# ---------------------------------------------------------------- device build

_BUILD_CACHE = {}


_LAST_BLOCKS = None


def _fix_prep_sems(nc, gather_order, gb):
    """Safety check (no lane-sem aliasing): the later of two same-lane
    gathers (8 apart in scheduled order) can only issue once its buffer's
    previous occupant (2*gb gathers back in program order) was consumed,
    i.e. that DMA completed and PE (serial, in window order) read it.  So
    same-lane pairs must be >= 2*gb apart in program order."""
    from concourse.tile_sem_assignment import PROC_NAME_TO_IDX
    base = PROC_NAME_TO_IDX["DMASW0"]
    nlanes = sum(1 for k in PROC_NAME_TO_IDX if k.startswith("DMASW"))
    sched = []
    for blk in nc.m.functions[0].blocks:
        for inst in blk.instructions:
            if type(inst).__name__ == "InstDMAGatherAnt":
                lane = inst.bass_scheduled_proc - base
                assert 0 <= lane < nlanes, (lane, inst.name)
                sched.append(inst.name)
    assert sorted(sched) == sorted(gather_order)
    prog = {n: i for i, n in enumerate(gather_order)}
    for i in range(len(sched) - nlanes):
        d = prog[sched[i + nlanes]] - prog[sched[i]]
        assert d >= 2 * gb, (
            f"same-lane gathers only {d} apart in program order (< 2*gb="
            f"{2 * gb}); lane sem could alias — lower gb")


def _build(blocks, e_tot, mode="full", nq=2, gb=2, hb=3, sb=4, scratch=65536,
           gwin=512, ib=3):
    key = (tuple(blocks), e_tot, mode, nq, gb, hb, sb, scratch, gwin, ib)
    if key in _BUILD_CACHE:
        return _BUILD_CACHE[key]
    do_gather = mode in ("full", "gather")
    do_compute = mode in ("full", "compute")

    nc = bacc.Bacc("TRN2", num_swdge_queues=nq,
                   dynamic_dma_scratch_size=scratch)
    dt = mybir.dt
    xs_t = nc.dram_tensor("xs", [N_NODES, IN_CH], dt.bfloat16, kind="ExternalInput")
    xi_t = nc.dram_tensor("xi", [N_NODES, IN_CH], dt.bfloat16, kind="ExternalInput")
    idx_fs = nc.dram_tensor("idx_fs", [128, e_tot // 16], dt.int16, kind="ExternalInput")
    idx_fi = nc.dram_tensor("idx_fi", [128, e_tot // 16], dt.int16, kind="ExternalInput")
    efT = nc.dram_tensor("efT", [EDGE_DIM + 1, e_tot], dt.bfloat16, kind="ExternalInput")
    offE = nc.dram_tensor("offE", [1, e_tot], dt.float32, kind="ExternalInput")
    # augmented W1 (lhsT): rows 0:128 fs, 128:192 ef, 192 b1, 193:321 fi
    w1 = nc.dram_tensor("w1", [2 * IN_CH + EDGE_DIM + 1, DEC_CH], dt.bfloat16,
                        kind="ExternalInput")
    w2 = nc.dram_tensor("w2", [DEC_CH], dt.bfloat16, kind="ExternalInput")
    out_d = nc.dram_tensor("out", [e_tot], dt.float32, kind="ExternalOutput")

    with tile.TileContext(nc) as tc, ExitStack() as ctx:
        const = ctx.enter_context(tc.tile_pool(name="const", bufs=1))
        gp = ctx.enter_context(tc.tile_pool(name="gp", bufs=2))
        ip = ctx.enter_context(tc.tile_pool(name="ip", bufs=ib))
        sp = ctx.enter_context(tc.tile_pool(name="sp", bufs=sb))
        op = ctx.enter_context(tc.tile_pool(name="op", bufs=2))
        hp = ctx.enter_context(tc.tile_pool(name="hp", bufs=hb, space="PSUM"))
        pp = ctx.enter_context(tc.tile_pool(name="pp", bufs=2, space="PSUM"))

        # weights: lhsT blocks [K, M] (K on partitions)
        w1_fs, w1_ef, w1_fi = [], [], []
        for m in range(2):
            ms = slice(m * 128, (m + 1) * 128)
            t = const.tile([128, 128], dt.bfloat16, name=f"w1fs{m}")
            nc.sync.dma_start(t[:], w1[0:128, ms])
            w1_fs.append(t)
            t = const.tile([65, 128], dt.bfloat16, name=f"w1ef{m}")
            nc.sync.dma_start(t[:], w1[128:193, ms])
            w1_ef.append(t)
            t = const.tile([128, 128], dt.bfloat16, name=f"w1fi{m}")
            nc.sync.dma_start(t[:], w1[193:321, ms])
            w1_fi.append(t)
        w2_sb = const.tile([128, 2], dt.bfloat16)
        for m in range(2):
            nc.sync.dma_start(w2_sb[:, m:m + 1], w2[m * 128:(m + 1) * 128])

        fs_c = fi_c = None
        if not do_gather:
            fs_c = const.tile([128, 1, gwin], dt.bfloat16, name="fs_c")
            nc.vector.memset(fs_c[:], 0.5)
            fi_c = const.tile([128, 1, gwin], dt.bfloat16, name="fi_c")
            nc.vector.memset(fi_c[:], 0.5)

        # Free-running gathers (no ordering deps) across nq SWDGE queues;
        # queue = build ordinal % nq.  nq is capped at 2: with >= 3 queues
        # concurrent desc-gens on >= 3 Q7 core pairs corrupt gathered data
        # (observed on HW at nq=3/4, clean at nq=1/2 — the desc-gen ucode
        # appears to double-buffer shared staging state, so two concurrent
        # desc-gens are safe but three are not).  In-flight gathers are
        # bounded to 2*gb consecutive ordinals by buffer-reuse WAR, which
        # keeps the 8 rotating DMASW completion-sem lanes alias-free
        # (checked post-scheduling in _fix_prep_sems).
        gather_order = []

        def gather(out_ap, src_ap, idx_ap, n_idx, elem):
            g = len(gather_order)
            q = g % nq
            inst = nc.gpsimd.dma_gather(
                out_ap, src_ap, idx_ap, n_idx, n_idx, elem,
                transpose=True, queue_num=q)
            gather_order.append(inst.ins.name)
            return inst

        for (a, n, bs, bd) in blocks:
            ifs = ip.tile([128, n // 16], dt.int16, tag="ifs")
            nc.sync.dma_start(ifs[:], idx_fs[:, a // 16:(a + n) // 16])
            ifi = ip.tile([128, n // 16], dt.int16, tag="ifi")
            nc.sync.dma_start(ifi[:], idx_fi[:, a // 16:(a + n) // 16])

            eft = gp.tile([EDGE_DIM + 1, n], dt.bfloat16, tag="ef")
            nc.sync.dma_start(eft[:], efT[:, a:a + n])

            oacc = offt = None
            if do_compute:
                offt = op.tile([1, n], dt.float32, tag="offt", name="offt")
                nc.sync.dma_start(offt[:], offE[:, a:a + n])
                oacc = op.tile([1, n], dt.float32, tag="oacc", name="oacc")

            for g0 in range(0, n, gwin):
                gn = min(gwin, n - g0)
                # batched gathers (gn idxs each) spread across the SWDGE
                # queues (Q7 core pairs) for parallel desc gen
                if do_gather:
                    fs_g = gp.tile([128, 1, gn], dt.bfloat16, tag="fs", bufs=gb)
                    fi_g = gp.tile([128, 1, gn], dt.bfloat16, tag="fi", bufs=gb)
                    gather(fs_g[:], xs_t[bs * BUCKET:, :],
                           ifs[:, g0 // 16:(g0 + gn) // 16], gn, IN_CH)
                    gather(fi_g[:], xi_t[bd * BUCKET:, :],
                           ifi[:, g0 // 16:(g0 + gn) // 16], gn, IN_CH)
                else:
                    fs_g, fi_g = fs_c, fi_c
                if not do_compute:
                    continue
                for w in range(g0, g0 + gn, WIN):
                    ws = slice(w, w + WIN)
                    gs = slice(w - g0, w - g0 + WIN)
                    rc = []
                    for m in range(2):
                        h = hp.tile([128, WIN], dt.float32, tag=f"h{m}", space="PSUM")
                        nc.tensor.matmul(h[:], w1_fs[m][:], fs_g[:, 0, gs], start=True, stop=False)
                        nc.tensor.matmul(h[:], w1_ef[m][:], eft[:, ws], start=False, stop=False)
                        nc.tensor.matmul(h[:], w1_fi[m][:], fi_g[:, 0, gs], start=False, stop=True)
                        e_t = sp.tile([128, WIN], dt.bfloat16, tag=f"e{m}")
                        nc.scalar.activation(e_t[:], h[:], mybir.ActivationFunctionType.Exp)
                        r_t = sp.tile([128, WIN], dt.bfloat16, tag=f"r{m}")
                        if m == 0:
                            nc.scalar.activation(r_t[:], h[:],
                                                 mybir.ActivationFunctionType.Relu)
                        else:
                            nc.vector.tensor_scalar(out=r_t[:], in0=h[:],
                                                    scalar1=0.0, scalar2=None,
                                                    op0=mybir.AluOpType.max)
                        c_t = sp.tile([128, WIN], dt.bfloat16, tag=f"c{m}")
                        nc.vector.tensor_scalar(out=c_t[:], in0=e_t[:],
                                                scalar1=1.0, scalar2=None,
                                                op0=mybir.AluOpType.min)
                        rc.append((r_t, c_t))

                    o_ps = pp.tile([1, WIN], dt.float32, tag="ops", space="PSUM")
                    nc.tensor.matmul(o_ps[:], w2_sb[:, 0:1], rc[0][0][:], start=True, stop=False)
                    nc.tensor.matmul(o_ps[:], w2_sb[:, 0:1], rc[0][1][:], start=False, stop=False)
                    nc.tensor.matmul(o_ps[:], w2_sb[:, 1:2], rc[1][0][:], start=False, stop=False)
                    nc.tensor.matmul(o_ps[:], w2_sb[:, 1:2], rc[1][1][:], start=False, stop=True)
                    nc.vector.tensor_add(oacc[0:1, ws], o_ps[:], offt[0:1, ws])

            if do_compute:
                nc.sync.dma_start(out_d[a:a + n], oacc[0:1, :])

    _fix_prep_sems(nc, gather_order, gb)
    nc.finalize()
    _BUILD_CACHE[key] = nc
    return nc


# ---------------------------------------------------------------- entry points

def prepare(x_student, x_item, edge_label_index, edge_feat, offset, W1, b1, W2, b2):
    """Host prep + program build. Returns (nc, in_maps, metas)."""
    src = np.asarray(edge_label_index[0], np.int64)
    dst = np.asarray(edge_label_index[1], np.int64)
    ef = np.asarray(edge_feat, np.float32)
    off = np.asarray(offset, np.float32).reshape(-1)

    blocks, e_tot, per_core = _prep_cores(src, dst, ef, off)

    xs_bf = np.asarray(x_student, np.float32).astype(BF16)
    xi_bf = np.asarray(x_item, np.float32).astype(BF16)

    w1_f = np.asarray(W1, np.float32)
    b1_f = np.asarray(b1, np.float32).reshape(-1)
    w1_aug = np.concatenate([w1_f[0:128], w1_f[128:192], b1_f[None, :],
                             w1_f[192:320]], axis=0).astype(BF16)
    w2_bf = np.asarray(W2, np.float32).reshape(-1).astype(BF16)
    # constant folded into the offset stream: b2 - sum(W2) (in bf16, as the
    # device accumulates), cancelling the min(exp,1)==1 rows exactly
    cst = float(np.asarray(b2, np.float32).reshape(-1)[0]
                - np.sum(w2_bf.astype(np.float32)))

    global _LAST_BLOCKS
    _LAST_BLOCKS = (blocks, e_tot)
    nc = _build(blocks, e_tot)
    in_maps = []
    for c in range(N_CORES):
        pc = per_core[c]
        in_maps.append({
            "xs": xs_bf, "xi": xi_bf,
            "idx_fs": pc["idx_fs"], "idx_fi": pc["idx_fi"],
            "efT": pc["efT"], "offE": (pc["offE"] + cst).astype(np.float32),
            "w1": w1_aug, "w2": w2_bf,
        })
    metas = [pc["pos"] for pc in per_core]
    return nc, in_maps, metas


def unshard(results, metas):
    out = np.empty((N_EDGES, 1), np.float32)
    for c in range(N_CORES):
        pos = metas[c]
        valid = pos >= 0
        part = np.empty(E_PER, np.float32)
        part[pos[valid]] = results[c]["out"][valid]
        out[c * E_PER:(c + 1) * E_PER, 0] = part
    return out


def kernel(x_student, x_item, edge_label_index, edge_feat, offset, W1, b1, W2, b2):
    nc, in_maps, metas = prepare(x_student, x_item, edge_label_index, edge_feat,
                                 offset, W1, b1, W2, b2)
    res = run_bass_kernel_spmd(nc, in_maps, core_ids=list(range(N_CORES)))
    return unshard(res.results, metas)
